# revision 67
# baseline (speedup 1.0000x reference)
"""GATv2 (2 layers) + mean-pool + linear head on 8 Trainium2 NeuronCores.

Sharding: destination nodes are range-partitioned across the 8 cores
(6250 nodes each, padded to 6272 = 49*128). Edges (with self-loops) are
sorted by destination and assigned to the owner of their dst. Per core:

  1. node transforms xl=x@Wl+b, xr=x@Wr-b for the local node slice (PE),
  2. AllGather of xl (source-side transform) so any core can gather any
     source row,
  3. per 128-dst tile: dma_gather of per-edge source rows (split in two
     index banks because gather indices are int16), per-edge scores via
     DVE/ACT, per-dst softmax denominator + weighted message aggregation
     via indicator matmuls on the PE (edges of a tile only reference the
     tile's 128 dsts), normalization folded into the psum read-out.

Softmax is computed without the segment-max shift (scores are O(1); the
shift cancels exactly) and per-dst score terms are dropped (they cancel
in the softmax too). leaky_relu(z) = relu(0.8 z) + 0.2 z with the 0.2*xr
part dropped (per-dst) and 0.2*xl kept.

Mean-pool uses an on-device one-hot(batch) indicator matmul per dst
tile; the 1/count scaling and the final linear head run on host.

Host/launch layout: all input-independent work (jax/axon init, Bass IR
build, NEFF compile, device warmup) happens at import time. kernel()
ships x as fp8_e4m3 (halves the dominant transfer; ~1e-3 output error
vs 2e-2 tolerance), bins the edges with one stable counting sort, and
dispatches every host->device transfer asynchronously (per-device
device_put with no client-side ack; the axon relay moves the bytes on
its own IO threads at ~70 MB/s aggregate) so the transfers overlap the
host-side preprocessing and the only blocking await in a call is the
final [8, 64] result fetch.
"""

import sys
import numpy as np

for _p in ("/opt/trn_rl_repo", "/root/.axon_site/_ro/trn_rl_repo"):
    if _p not in sys.path:
        sys.path.insert(0, _p)

import ml_dtypes

BF = ml_dtypes.bfloat16
F8 = ml_dtypes.float8_e4m3

# Problem constants
N, E, F_IN, H, C, G = 50000, 800000, 128, 4, 64, 8
HC = H * C                      # 256
NCORES = 8
RP = N // NCORES                # 6250 rows per core
RPAD = 6272                     # 49*128
NT = RPAD // 128                # 49 dst/node tiles per core
NPADG = NCORES * RPAD           # 50176 padded global rows
BANKA = 5 * RPAD                # 31360; int16 gather bank split
PAD = RPAD - RP                 # 22 pad rows per core

# Deterministic for the fixed-seed reference graph; rebuilt on overflow.
CHA0, CHB0 = 12, 8


def _woffs():
    """Column offsets inside the bf16 weight pack (input-independent)."""
    o = {}
    c = 0
    for name, w in (("wl1", HC), ("wr1", HC), ("wl2", 2 * C), ("wr2", 2 * C),
                    ("att1", HC), ("att2", C), ("b1", HC), ("b2", C)):
        o[name] = (c, c + w)
        c += w
    o["_total"] = c
    return o


WOFF = _woffs()


# ----------------------------------------------------------------- host prep

def _preprocess(edge_index, batch, CHA, CHB):
    """One global counting sort of the 850k edges into (core, tile, bank)
    groups, then vectorized scatter into the packed device layouts."""
    CH = CHA + CHB
    nA, nB = CHA * 128, CHB * 128
    loop = np.arange(N, dtype=np.int32)
    src = np.concatenate([edge_index[0].astype(np.int32), loop])
    dst = np.concatenate([edge_index[1].astype(np.int32), loop])
    sp = src + PAD * (src // RP)           # padded global src row
    core = dst // RP
    dloc = dst - core * RP                 # 0..6249 local dst row
    coret = core * NT + (dloc >> 7)        # core*NT + tile
    bank = (sp >= BANKA).astype(np.int32)
    g = (coret * 2 + bank).astype(np.int16)
    cnt = np.bincount(g, minlength=2 * NCORES * NT)
    if cnt[0::2].max() > nA or cnt[1::2].max() > nB:
        return None  # overflow; caller retries with bigger capacity
    order = np.argsort(g, kind="stable")
    gs = g[order]
    sps = sp[order]
    dlocs = dloc[order]
    starts = np.concatenate([[0], np.cumsum(cnt)[:-1]]).astype(np.int32)
    pos = np.arange(len(gs), dtype=np.int32) - starts[gs]
    ct = gs >> 1

    NTG = NCORES * NT
    idx = np.zeros((NTG, CH * 128), np.int16)     # bankA cols | bankB cols
    dlp = np.full((NTG, CH * 128), -1, np.int8)
    bank_s = gs & 1
    col = pos + bank_s * nA                       # bank B starts at col nA
    idx[ct, col] = (sps - bank_s * BANKA).astype(np.int16)
    dlp[ct, col] = (dlocs & 127).astype(np.int8)  # dst row within tile
    # per-edge xr gather rows are derived on device from dl (tile-local)

    def wrap16(a):   # [NCORES*NT, n] -> [NCORES, 16, NT*n//16]
        n = a.shape[1]
        return (a.reshape(NCORES, NT, n // 16, 16).transpose(0, 3, 1, 2)
                .reshape(NCORES, 16, NT * n // 16))

    pidx = np.concatenate(
        [wrap16(idx[:, :nA]), wrap16(idx[:, nA:])],
        axis=2).reshape(NCORES * 16, -1)

    # pdl pack: dl columns then batch columns, [128, NT*CH + NT] int8 per
    # core, byte-viewed as [16, (NT*CH+NT)*8] int16 and merged with pidx
    # into a single per-core transfer buffer
    DLC = NT * CH + NT
    pdl = np.empty((NCORES, 128, DLC), np.int8)
    pdl[:, :, :NT * CH] = (dlp.reshape(NCORES, NT, CH, 128)
                           .transpose(0, 3, 1, 2).reshape(NCORES, 128, NT * CH))
    bpad = np.zeros((NCORES, RPAD), np.int8)
    bpad[:, :RP] = batch.reshape(NCORES, RP)
    pdl[:, :, NT * CH:] = bpad.reshape(NCORES, NT, 128).transpose(0, 2, 1)
    cntg = np.bincount(batch.astype(np.int64), minlength=G).astype(np.float32)
    pmix = np.concatenate(
        [pidx.reshape(NCORES, -1),
         pdl.reshape(NCORES, -1).view(np.int16)], axis=1)
    return pmix, cntg


# ---------------------------------------------------------------- bass build

def _build_nc(CHA, CHB):
    from contextlib import ExitStack
    from concourse import bacc, mybir
    from concourse import tile

    F32 = mybir.dt.float32
    BF16 = mybir.dt.bfloat16
    FP8 = mybir.dt.float8e4
    I16 = mybir.dt.int16
    AF = mybir.ActivationFunctionType
    OP = mybir.AluOpType
    CH = CHA + CHB
    CI = NT * (CHA + CHB) * 8

    nc = bacc.Bacc(None, target_bir_lowering=False, debug=False)
    dp = nc.declare_dram_parameter
    I8 = mybir.dt.int8
    DLC = NT * CH + NT
    px8 = dp("px8", [128, RPAD], FP8, isOutput=False)
    pw = dp("pw", [16, WOFF["_total"]], BF16, isOutput=False)
    # single flat per-core buffer: [16, CI] int16 gather indices followed
    # by the [128, DLC] int8 dl/batch pack, both row-major
    pmix = dp("pmix", [1, 16 * CI + 64 * DLC], I16, isOutput=False)
    pidx = pmix[:, 0:16 * CI].rearrange("o (p c) -> (o p) c", p=16)
    pdl = (pmix[:, 16 * CI:].bitcast(I8)
           .rearrange("o (p c) -> (o p) c", p=128))
    pw_loc = nc.dram_tensor("pw_loc", [16, WOFF["_total"]], BF16)
    pw_full = nc.dram_tensor("pw_full", [128, WOFF["_total"]], BF16,
                             addr_space="Shared")
    out_pool = dp("out_pool", [G, C], F32, isOutput=True)

    xl1_sl = nc.dram_tensor("xl1_sl", [RPAD, HC], BF16)
    xr1_loc = nc.dram_tensor("xr1_loc", [RPAD, HC], BF16)
    xl1_full = nc.dram_tensor("xl1_full", [NPADG, HC], BF16, addr_space="Shared")
    h1c_sl = [nc.dram_tensor(f"h1c{i}_sl", [RPAD, 128], BF16) for i in range(2)]
    xl2_sl = nc.dram_tensor("xl2_sl", [RPAD, C], F32)
    xr2_loc = nc.dram_tensor("xr2_loc", [RPAD, C], F32)
    xl2_full = nc.dram_tensor("xl2_full", [NPADG, C], F32, addr_space="Shared")
    pool_part = nc.dram_tensor("pool_part", [G, C], F32)
    pool_sum = nc.dram_tensor("pool_sum", [G, C], F32, addr_space="Shared")

    with tile.TileContext(nc) as tc, ExitStack() as ctx:
        cp = ctx.enter_context(tc.tile_pool(name="consts", bufs=1))
        sb = ctx.enter_context(tc.tile_pool(name="work", bufs=2))

        # weights ship as a per-core 16-row shard; device AllGather
        # reassembles the full [128, W] pack (0.36 MB over the host wire
        # instead of 2.9 MB of per-core replicas). Collectives cannot read
        # IO tensors, so bounce the shard through an internal DRAM tensor.
        pwt = cp.tile([16, WOFF["_total"]], BF16, tag="pwt")
        nc.sync.dma_start(pwt[:], pw[:, :])
        nc.sync.dma_start(pw_loc[:, :], pwt[:])
        nc.gpsimd.collective_compute(
            "AllGather", mybir.AluOpType.bypass,
            replica_groups=[list(range(NCORES))],
            ins=[pw_loc[:, :]], outs=[pw_full[:, :]])

        def cload(name, lo, hi, dtype=BF16):
            t = cp.tile([128, hi - lo], dtype, tag=name)
            nc.sync.dma_start(t[:], pw_full[:, lo:hi])
            return t

        x8 = cp.tile([128, RPAD], FP8, tag="x8c")
        nc.sync.dma_start(x8[:], px8[:, :])
        wl1_t = cload("wl1c", *WOFF["wl1"])
        wr1_t = cload("wr1c", *WOFF["wr1"])
        wl2_t = cload("wl2c", *WOFF["wl2"])          # [128, 2*C]
        wr2_t = cload("wr2c", *WOFF["wr2"])
        att_t = cload("attc", *WOFF["att1"])
        att2_t = cload("att2c", *WOFF["att2"])
        b1_b = cload("b1c", *WOFF["b1"])
        b2_b = cload("b2c", *WOFF["b2"])
        dl_t = cp.tile([128, NT * CH], I8, tag="dlc")
        nc.sync.dma_start(dl_t[:], pdl[:, :NT * CH])
        batch_t = cp.tile([128, NT], I8, tag="batchc")
        nc.sync.dma_start(batch_t[:], pdl[:, NT * CH:])

        b1_t = cp.tile([128, HC], F32)
        nc.vector.tensor_copy(b1_t[:], b1_b[:])
        b2_t = cp.tile([128, C], F32)
        nc.vector.tensor_copy(b2_t[:], b2_b[:])

        iota_i = cp.tile([128, 128], mybir.dt.int32)
        nc.gpsimd.iota(iota_i[:], pattern=[[1, 128]], base=0, channel_multiplier=0)
        iota_f = cp.tile([128, 128], I8)
        nc.vector.tensor_copy(iota_f[:], iota_i[:])

        nIA, nIB = NT * CHA * 8, NT * CHB * 8
        iA_t = cp.tile([128, nIA], I16)
        iB_t = cp.tile([128, nIB], I16)
        for k in range(8):
            nc.sync.dma_start(iA_t[16 * k:16 * (k + 1), :], pidx[:, 0:nIA])
            nc.sync.dma_start(iB_t[16 * k:16 * (k + 1), :], pidx[:, nIA:nIA + nIB])

        # xr gather rows, derived from dl on device: clamp(-1 -> 0), cast to
        # int16, then shuffle into the 16-row-wrap dma_gather index layout
        # (row p%16, col chunk*8 + p//16) and replicate into all 8 row groups.
        dl0 = cp.tile([128, NT * CH], I8)
        nc.vector.tensor_scalar(dl0[:], dl_t[:], 0, None, OP.max)
        dli = cp.tile([128, NT * CH], I16)
        nc.vector.tensor_copy(dli[:], dl0[:])
        iR3 = cp.tile([128, NT * CH, 8], I16)
        for q in range(8):
            nc.sync.dma_start(iR3[0:16, :, q], dli[16 * q:16 * (q + 1), :])
        for k in range(1, 8):
            nc.sync.dma_start(iR3[16 * k:16 * (k + 1), :, :], iR3[0:16, :, :])
        iR_t = iR3[:].rearrange("p a b -> p (a b)")

        # ---- stage 1: layer-1 node transforms (x fp8 -> bf16 per tile)
        psx = ExitStack()
        ctx.callback(psx.close)
        ps = psx.enter_context(tc.tile_pool(name="ps1", bufs=2, space="PSUM"))
        for i in range(NT):
            xs = sb.tile([128, 128], BF16, tag="xs")
            nc.vector.tensor_copy(xs[:], x8[:, i * 128:(i + 1) * 128])
            pa = ps.tile([128, HC], F32, tag="p_nl")
            nc.tensor.matmul(pa[:], xs[:], wl1_t[:], start=True, stop=True)
            ta = sb.tile([128, HC], BF16, tag="t_nl")
            nc.vector.tensor_add(ta[:], pa[:], b1_t[:])
            nc.sync.dma_start(xl1_sl[i * 128:(i + 1) * 128, :], ta[:])
            pb = ps.tile([128, HC], F32, tag="p_nr")
            nc.tensor.matmul(pb[:], xs[:], wr1_t[:], start=True, stop=True)
            tb = sb.tile([128, HC], BF16, tag="t_nr")
            nc.vector.tensor_sub(tb[:], pb[:], b1_t[:])
            nc.sync.dma_start(xr1_loc[i * 128:(i + 1) * 128, :], tb[:])

        nc.gpsimd.collective_compute(
            "AllGather", mybir.AluOpType.bypass,
            replica_groups=[list(range(NCORES))],
            ins=[xl1_sl[:, :]], outs=[xl1_full[:, :]])

        MAXCH = 8  # dma_gather is only safe up to 1024 indices per call

        def gathers(out3, in_ap, idx_t, col0, nch, elem):
            for b0 in range(0, nch, MAXCH):
                b1 = min(b0 + MAXCH, nch)
                n = (b1 - b0) * 128
                nc.gpsimd.dma_gather(
                    out3[:, b0:b1, :], in_ap,
                    idx_t[:, col0 + b0 * 8:col0 + b1 * 8],
                    num_idxs=n, num_idxs_reg=n, elem_size=elem)

        # ---- stage 2: layer-1 edge stage per dst tile
        psx.close()
        ps = psx.enter_context(tc.tile_pool(name="ps2", bufs=2, space="PSUM"))
        for t in range(NT):
            gxl = sb.tile([128, CH, HC], BF16, tag="gxl")
            gathers(gxl[:, 0:CHA, :], xl1_full[0:BANKA, :], iA_t,
                    t * CHA * 8, CHA, HC)
            gathers(gxl[:, CHA:CH, :], xl1_full[BANKA:NPADG, :], iB_t,
                    t * CHB * 8, CHB, HC)
            gxr = sb.tile([128, CH, HC], BF16, tag="gxr")
            gathers(gxr[:, :, :], xr1_loc[t * 128:(t + 1) * 128, :], iR_t,
                    t * CH * 8, CH, HC)

            ex_t = sb.tile([128, CH, H], BF16, tag="ex")
            ind_t = sb.tile([128, CH, 128], BF16, tag="ind")
            den_p = ps.tile([128, H], F32, tag="den")
            z = sb.tile([128, CH, HC], BF16, tag="z", bufs=1)
            nc.vector.tensor_add(z[:], gxl[:], gxr[:])
            r = sb.tile([128, CH, HC], BF16, tag="r", bufs=1)
            nc.scalar.activation(r[:], z[:], AF.Relu, scale=0.8)
            nc.scalar.mul(z[:], gxl[:], 0.2)
            nc.vector.tensor_add(r[:], r[:], z[:])
            nc.vector.tensor_tensor(
                z[:], r[:],
                att_t[:].unsqueeze(1).broadcast_to([128, CH, HC]), OP.mult)
            sc = sb.tile([128, CH, H], F32, tag="sc")
            nc.vector.tensor_reduce(
                sc[:], z[:].rearrange("p t (h c) -> p t h c", h=H),
                axis=mybir.AxisListType.X, op=OP.add)
            nc.scalar.activation(ex_t[:], sc[:], AF.Exp)
            nc.vector.tensor_tensor(
                ind_t[:], iota_f[:].unsqueeze(1).broadcast_to([128, CH, 128]),
                dl_t[:, t * CH:(t + 1) * CH].unsqueeze(2).broadcast_to(
                    [128, CH, 128]), OP.is_equal)
            for j in range(CH):
                nc.tensor.matmul(den_p[:], ind_t[:, j, :], ex_t[:, j, :],
                                 start=(j == 0), stop=(j == CH - 1))
            rden = sb.tile([128, H], F32, tag="rden")
            nc.vector.tensor_scalar(rden[:], den_p[:], 1e-20, None, OP.max)
            nc.vector.reciprocal(rden[:], rden[:])

            agg_p = ps.tile([128, HC], F32, tag="agg")
            msg = sb.tile([128, CH, HC], BF16, tag="msg", bufs=1)
            nc.vector.tensor_tensor(
                msg[:].rearrange("p t (h c) -> p t h c", h=H),
                gxl[:].rearrange("p t (h c) -> p t h c", h=H),
                ex_t[:].unsqueeze(3).broadcast_to([128, CH, H, C]), OP.mult)
            for j in range(CH):
                nc.tensor.matmul(agg_p[:], ind_t[:, j, :], msg[:, j, :],
                                 start=(j == 0), stop=(j == CH - 1))
            h1_t = sb.tile([128, HC], BF16, tag="h1")
            for h in range(H):
                nc.scalar.activation(h1_t[:, h * C:(h + 1) * C],
                                     agg_p[:, h * C:(h + 1) * C],
                                     AF.Relu, scale=rden[:, h:h + 1])
            for i in range(2):
                nc.sync.dma_start(h1c_sl[i][t * 128:(t + 1) * 128, :],
                                  h1_t[:, i * 128:(i + 1) * 128])

        # ---- stage 3: layer-2 node transforms
        psx.close()
        ps = psx.enter_context(tc.tile_pool(name="ps3", bufs=2, space="PSUM"))
        h1T = cp.tile([128, 2, RPAD], BF16)
        for i in range(2):
            nc.sync.dma_start_transpose(h1T[:, i, :], h1c_sl[i][:, :])
        for i in range(NT):
            pa = ps.tile([128, C], F32, tag="p2_nl")
            for cc in range(2):
                nc.tensor.matmul(pa[:], h1T[:, cc, i * 128:(i + 1) * 128],
                                 wl2_t[:, cc * C:(cc + 1) * C],
                                 start=(cc == 0), stop=(cc == 1))
            ta = sb.tile([128, C], F32, tag="t2_nl")
            nc.vector.tensor_add(ta[:], pa[:], b2_t[:])
            nc.sync.dma_start(xl2_sl[i * 128:(i + 1) * 128, :], ta[:])
            pb = ps.tile([128, C], F32, tag="p2_nr")
            for cc in range(2):
                nc.tensor.matmul(pb[:], h1T[:, cc, i * 128:(i + 1) * 128],
                                 wr2_t[:, cc * C:(cc + 1) * C],
                                 start=(cc == 0), stop=(cc == 1))
            tb = sb.tile([128, C], F32, tag="t2_nr")
            nc.vector.tensor_sub(tb[:], pb[:], b2_t[:])
            nc.sync.dma_start(xr2_loc[i * 128:(i + 1) * 128, :], tb[:])

        nc.gpsimd.collective_compute(
            "AllGather", mybir.AluOpType.bypass,
            replica_groups=[list(range(NCORES))],
            ins=[xl2_sl[:, :]], outs=[xl2_full[:, :]])

        # ---- stage 4: layer-2 edge stage + pooling
        psx.close()
        ps = psx.enter_context(tc.tile_pool(name="ps4", bufs=2, space="PSUM"))
        pool_acc = cp.tile([G, C], F32)
        nc.vector.memset(pool_acc[:], 0.0)
        for t in range(NT):
            gxl2 = sb.tile([128, CH, C], F32, tag="gxl2")
            gathers(gxl2[:, 0:CHA, :], xl2_full[0:BANKA, :], iA_t,
                    t * CHA * 8, CHA, C)
            gathers(gxl2[:, CHA:CH, :], xl2_full[BANKA:NPADG, :], iB_t,
                    t * CHB * 8, CHB, C)
            gxr2 = sb.tile([128, CH, C], F32, tag="gxr2")
            gathers(gxr2[:, :, :], xr2_loc[t * 128:(t + 1) * 128, :], iR_t,
                    t * CH * 8, CH, C)

            ex2_t = sb.tile([128, CH, 1], BF16, tag="ex2")
            ind2_t = sb.tile([128, CH, 128], BF16, tag="ind2")
            den2_p = ps.tile([128, 1], F32, tag="den2")
            z = sb.tile([128, CH, C], BF16, tag="z2", bufs=1)
            nc.vector.tensor_add(z[:], gxl2[:], gxr2[:])
            r = sb.tile([128, CH, C], BF16, tag="r2", bufs=1)
            nc.scalar.activation(r[:], z[:], AF.Relu, scale=0.8)
            nc.scalar.mul(z[:], gxl2[:], 0.2)
            nc.vector.tensor_add(r[:], r[:], z[:])
            nc.vector.tensor_tensor(
                z[:], r[:],
                att2_t[:].unsqueeze(1).broadcast_to([128, CH, C]), OP.mult)
            sc = sb.tile([128, CH, 1], F32, tag="sc2")
            nc.vector.tensor_reduce(
                sc[:], z[:].unsqueeze(2),
                axis=mybir.AxisListType.X, op=OP.add)
            nc.scalar.activation(ex2_t[:], sc[:], AF.Exp)
            nc.vector.tensor_tensor(
                ind2_t[:], iota_f[:].unsqueeze(1).broadcast_to([128, CH, 128]),
                dl_t[:, t * CH:(t + 1) * CH].unsqueeze(2).broadcast_to(
                    [128, CH, 128]), OP.is_equal)
            for j in range(CH):
                nc.tensor.matmul(den2_p[:], ind2_t[:, j, :], ex2_t[:, j, :],
                                 start=(j == 0), stop=(j == CH - 1))
            rden2 = sb.tile([128, 1], F32, tag="rden2")
            nc.vector.tensor_scalar(rden2[:], den2_p[:], 1e-20, None, OP.max)
            nc.vector.reciprocal(rden2[:], rden2[:])

            agg2_p = ps.tile([128, C], F32, tag="agg2")
            msg = sb.tile([128, CH, C], BF16, tag="msg2", bufs=1)
            nc.vector.tensor_tensor(
                msg[:], gxl2[:],
                ex2_t[:].broadcast_to([128, CH, C]), OP.mult)
            for j in range(CH):
                nc.tensor.matmul(agg2_p[:], ind2_t[:, j, :], msg[:, j, :],
                                 start=(j == 0), stop=(j == CH - 1))
            h2_t = sb.tile([128, C], BF16, tag="h2")
            nc.scalar.mul(h2_t[:], agg2_p[:], rden2[:, 0:1])

            indp = sb.tile([128, G], BF16, tag="indp")
            nc.vector.tensor_tensor(
                indp[:], iota_f[:, 0:G],
                batch_t[:, t:t + 1].broadcast_to([128, G]), OP.is_equal)
            pool_p = ps.tile([G, C], F32, tag="poolp")
            nc.tensor.matmul(pool_p[:], indp[:], h2_t[:],
                             start=True, stop=True)
            nc.vector.tensor_add(pool_acc[:], pool_acc[:], pool_p[:])

        ot = cp.tile([G, C], F32)
        nc.vector.tensor_copy(ot[:], pool_acc[:])
        nc.sync.dma_start(pool_part[:, :], ot[:])
        nc.gpsimd.collective_compute(
            "AllReduce", mybir.AluOpType.add,
            replica_groups=[list(range(NCORES))],
            ins=[pool_part[:, :]], outs=[pool_sum[:, :]])
        nc.sync.dma_start(out_pool[:, :], pool_sum[:, :])

    nc.finalize()
    return nc


# ------------------------------------------------------------ cached runner

class _Runner:
    """Holds the Bass module, a persistently-jitted shard_map callable,
    and the device mesh, so repeat kernel() calls skip all tracing,
    lowering, and compilation."""

    def __init__(self, CHA, CHB):
        import jax
        from jax.sharding import Mesh, PartitionSpec, NamedSharding
        from jax.experimental.shard_map import shard_map
        from concourse import bass2jax, mybir

        self.jax = jax
        self.CHA, self.CHB = CHA, CHB
        nc = _build_nc(CHA, CHB)
        bass2jax.install_neuronx_cc_hook()

        partition_name = (nc.partition_id_tensor.name
                          if nc.partition_id_tensor else None)
        in_names, out_names, out_avals, zero_shapes = [], [], [], []
        for alloc in nc.m.functions[0].allocations:
            if not isinstance(alloc, mybir.MemoryLocationSet):
                continue
            name = alloc.memorylocations[0].name
            if alloc.kind == "ExternalInput":
                if name != partition_name:
                    in_names.append(name)
            elif alloc.kind == "ExternalOutput":
                shape = tuple(alloc.tensor_shape)
                dtype = mybir.dt.np(alloc.dtype)
                out_names.append(name)
                out_avals.append(jax.core.ShapedArray(shape, dtype))
                zero_shapes.append((shape, dtype))
        n_params = len(in_names)
        all_names = list(in_names) + list(out_names)
        if partition_name is not None:
            all_names.append(partition_name)

        def _body(*args):
            operands = list(args)
            if partition_name is not None:
                operands.append(bass2jax.partition_id_tensor())
            outs = bass2jax._bass_exec_p.bind(
                *operands,
                out_avals=tuple(out_avals),
                in_names=tuple(all_names),
                out_names=tuple(out_names),
                lowering_input_output_aliases=(),
                sim_require_finite=True,
                sim_require_nnan=True,
                nc=nc,
            )
            return tuple(outs)

        self.devices = jax.devices()[:NCORES]
        assert len(self.devices) == NCORES
        mesh = Mesh(np.asarray(self.devices), ("core",))
        self.sharding = NamedSharding(mesh, PartitionSpec("core"))
        n_outs = len(out_names)
        # No donation: the kernel writes every output element, so the
        # uninitialized custom-call result buffers are fine, and the zero
        # "output-seed" inputs can live on device permanently.
        self.fn = jax.jit(
            shard_map(_body, mesh=mesh,
                      in_specs=(PartitionSpec("core"),) * (n_params + n_outs),
                      out_specs=(PartitionSpec("core"),) * n_outs,
                      check_rep=False),
            keep_unused=True)
        self.in_names = in_names
        self.out_names = out_names
        self.zero_shapes = zero_shapes
        self.dev_zeros = [
            self.start_put(np.zeros((NCORES * s[0], *s[1:]), d))()
            for s, d in zero_shapes]
        # XLA-vectorized f32->fp8 cast on the CPU backend: ~6ms for x vs
        # ~52ms through ml_dtypes' scalar loop
        import jax.numpy as jnp
        self.cast8 = jax.jit(lambda a: a.astype(jnp.float8_e4m3),
                             backend="cpu")
        np.asarray(self.cast8(np.zeros((N, F_IN), np.float32)))

    def start_put(self, arr=None, shape=None, dtype=None, produce=None):
        """Async per-device sharded transfer: device_put dispatch only (the
        relay's own IO threads move the bytes), global array assembled from
        the unready buffers. No client-side ack round-trip — execution is
        sequenced after the transfers server-side; the only blocking await
        in a call is the final result fetch. Returns a handle for symmetry
        with the old threaded API."""
        jax = self.jax
        if arr is not None:
            shape = arr.shape
            d0 = shape[0] // NCORES
            per = arr.reshape(NCORES, d0, *shape[1:])
            produce = lambda i: per[i]
        bufs = [jax.device_put(np.ascontiguousarray(produce(i)),
                               self.devices[i])
                for i in range(NCORES)]
        garr = jax.make_array_from_single_device_arrays(
            shape, self.sharding, bufs)
        return lambda: garr

    def run_handles(self, handles):
        """handles: name -> handle from start_put. Returns the [G, C]
        pooled sum (identical on every core after the on-device AllReduce;
        only core 0's shard is pulled back)."""
        args = [handles[n]() for n in self.in_names]
        outs = self.fn(*args, *self.dev_zeros)
        return np.asarray(outs[0].addressable_shards[0].data)


_RUNNERS = {}


def _get_runner(CHA, CHB):
    key = (CHA, CHB)
    if key not in _RUNNERS:
        _RUNNERS[key] = _Runner(CHA, CHB)
    return _RUNNERS[key]


def _warmup():
    r = _get_runner(CHA0, CHB0)
    CH = CHA0 + CHB0
    CI = NT * (CHA0 + CHB0) * 8
    handles = {
        "px8": r.start_put(np.zeros((NCORES * 128, RPAD), F8)),
        "pw": r.start_put(np.zeros((NCORES * 16, WOFF["_total"]), BF)),
        "pmix": r.start_put(np.zeros(
            (NCORES, 16 * CI + 64 * (NT * CH + NT)), np.int16)),
    }
    r.run_handles(handles)
    return r


try:
    _warmup()
except Exception:
    _RUNNERS.clear()


# -------------------------------------------------------------------- driver

def kernel(x, edge_index, batch, Wl1, Wr1, att1, b1, Wl2, Wr2, att2, b2,
           Wo, bo):
    x = np.asarray(x, np.float32)
    edge_index = np.asarray(edge_index)
    batch = np.asarray(batch)
    Wl1 = np.asarray(Wl1, np.float32); Wr1 = np.asarray(Wr1, np.float32)
    att1 = np.asarray(att1, np.float32); b1 = np.asarray(b1, np.float32)
    Wl2 = np.asarray(Wl2, np.float32); Wr2 = np.asarray(Wr2, np.float32)
    att2 = np.asarray(att2, np.float32); b2 = np.asarray(b2, np.float32)
    Wo = np.asarray(Wo, np.float32); bo = np.asarray(bo, np.float32)

    CHA, CHB = CHA0, CHB0
    runner = _RUNNERS.get((CHA, CHB)) or _get_runner(CHA, CHB)

    # weight pack is tiny and preprocessing-independent: fill + ship first
    # so the wire is busy during the fp8 cast of x
    pwh = np.empty((128, WOFF["_total"]), BF)

    def put(name, a):
        lo, hi = WOFF[name]
        pwh[:, lo:hi] = a.astype(BF)

    put("wl1", Wl1); put("wr1", Wr1)
    put("wl2", Wl2.reshape(2, 128, C).transpose(1, 0, 2).reshape(128, 2 * C))
    put("wr2", Wr2.reshape(2, 128, C).transpose(1, 0, 2).reshape(128, 2 * C))
    put("att1", np.broadcast_to(att1.reshape(1, HC), (128, HC)))
    put("att2", np.broadcast_to(att2.reshape(1, C), (128, C)))
    put("b1", np.broadcast_to(b1.reshape(1, HC), (128, HC)))
    put("b2", np.broadcast_to(b2.reshape(1, C), (128, C)))
    h_w = runner.start_put(pwh)   # [128, W]: 16-row shard per core

    # x: one XLA-vectorized fp8 cast (~6ms), then per-core byte transposes
    # into the padded shards; each core's transfer dispatches immediately
    y8 = np.asarray(runner.cast8(x)).reshape(NCORES, RP, F_IN)

    def make_px8(i):
        b = np.empty((128, RPAD), F8)
        b[:, :RP] = y8[i].T
        b[:, RP:] = 0.0
        return b

    h_x = runner.start_put(shape=(NCORES * 128, RPAD), dtype=F8,
                           produce=make_px8)

    pre = _preprocess(edge_index, batch, CHA, CHB)
    while pre is None:  # capacity overflow: grow and rebuild (cold path)
        CHA += 2; CHB += 2
        pre = _preprocess(edge_index, batch, CHA, CHB)
        runner = _get_runner(CHA, CHB)
    pmix, cntg = pre
    h_mix = runner.start_put(pmix)

    pooled = runner.run_handles(dict(px8=h_x, pw=h_w, pmix=h_mix))
    pooled = pooled / np.maximum(cntg, 1.0)[:, None]
    return (pooled @ Wo + bo).astype(np.float32)


# revision 69
# speedup vs baseline: 1.0115x; 1.0115x over previous
"""GATv2 (2 layers) + mean-pool + linear head on 8 Trainium2 NeuronCores.

Sharding: destination nodes are range-partitioned across the 8 cores
(6250 nodes each, padded to 6272 = 49*128). Edges (with self-loops) are
sorted by destination and assigned to the owner of their dst. Per core:

  1. node transforms xl=x@Wl+b, xr=x@Wr-b for the local node slice (PE),
  2. AllGather of xl (source-side transform) so any core can gather any
     source row,
  3. per 128-dst tile: dma_gather of per-edge source rows (split in two
     index banks because gather indices are int16), per-edge scores via
     DVE/ACT, per-dst softmax denominator + weighted message aggregation
     via indicator matmuls on the PE (edges of a tile only reference the
     tile's 128 dsts), normalization folded into the psum read-out.

Softmax is computed without the segment-max shift (scores are O(1); the
shift cancels exactly) and per-dst score terms are dropped (they cancel
in the softmax too). leaky_relu(z) = relu(0.8 z) + 0.2 z with the 0.2*xr
part dropped (per-dst) and 0.2*xl kept.

Mean-pool uses an on-device one-hot(batch) indicator matmul per dst
tile; the 1/count scaling and the final linear head run on host.

Host/launch layout: all input-independent work (jax/axon init, Bass IR
build, NEFF compile, device warmup) happens at import time. kernel()
ships x as fp8_e4m3 (halves the dominant transfer; ~1e-3 output error
vs 2e-2 tolerance), bins the edges with one stable counting sort, and
dispatches every host->device transfer asynchronously (per-device
device_put with no client-side ack; the axon relay moves the bytes on
its own IO threads at ~70 MB/s aggregate) so the transfers overlap the
host-side preprocessing and the only blocking await in a call is the
final [8, 64] result fetch.
"""

import sys
import numpy as np

for _p in ("/opt/trn_rl_repo", "/root/.axon_site/_ro/trn_rl_repo"):
    if _p not in sys.path:
        sys.path.insert(0, _p)

import ml_dtypes

BF = ml_dtypes.bfloat16
F8 = ml_dtypes.float8_e4m3

# Problem constants
N, E, F_IN, H, C, G = 50000, 800000, 128, 4, 64, 8
HC = H * C                      # 256
NCORES = 8
RP = N // NCORES                # 6250 rows per core
RPAD = 6272                     # 49*128
NT = RPAD // 128                # 49 dst/node tiles per core
NPADG = NCORES * RPAD           # 50176 padded global rows
BANKA = 5 * RPAD                # 31360; int16 gather bank split
PAD = RPAD - RP                 # 22 pad rows per core

# Deterministic for the fixed-seed reference graph; rebuilt on overflow.
CHA0, CHB0 = 12, 8


def _woffs():
    """Column offsets inside the bf16 weight pack (input-independent)."""
    o = {}
    c = 0
    for name, w in (("wl1", HC), ("wr1", HC), ("wl2", 2 * C), ("wr2", 2 * C),
                    ("att1", HC), ("att2", C), ("b1", HC), ("b2", C)):
        o[name] = (c, c + w)
        c += w
    o["_total"] = c
    return o


WOFF = _woffs()


# ----------------------------------------------------------------- host prep

def _preprocess(edge_index, batch, CHA, CHB):
    """One global counting sort of the 850k edges into (core, tile, bank)
    groups, then vectorized scatter into the packed device layouts."""
    CH = CHA + CHB
    nA, nB = CHA * 128, CHB * 128
    loop = np.arange(N, dtype=np.int32)
    src = np.concatenate([edge_index[0].astype(np.int32), loop])
    dst = np.concatenate([edge_index[1].astype(np.int32), loop])
    sp = src + PAD * (src // RP)           # padded global src row
    core = dst // RP
    dloc = dst - core * RP                 # 0..6249 local dst row
    coret = core * NT + (dloc >> 7)        # core*NT + tile
    bank = (sp >= BANKA).astype(np.int32)
    g = (coret * 2 + bank).astype(np.int16)
    cnt = np.bincount(g, minlength=2 * NCORES * NT)
    if cnt[0::2].max() > nA or cnt[1::2].max() > nB:
        return None  # overflow; caller retries with bigger capacity
    order = np.argsort(g, kind="stable")
    gs = g[order]
    sps = sp[order]
    dlocs = dloc[order]
    starts = np.concatenate([[0], np.cumsum(cnt)[:-1]]).astype(np.int32)
    pos = np.arange(len(gs), dtype=np.int32) - starts[gs]
    ct = gs >> 1

    NTG = NCORES * NT
    idx = np.zeros((NTG, CH * 128), np.int16)     # bankA cols | bankB cols
    dlp = np.full((NTG, CH * 128), -1, np.int8)
    bank_s = gs & 1
    col = pos + bank_s * nA                       # bank B starts at col nA
    idx[ct, col] = (sps - bank_s * BANKA).astype(np.int16)
    dlp[ct, col] = (dlocs & 127).astype(np.int8)  # dst row within tile
    # per-edge xr gather rows are derived on device from dl (tile-local)

    def wrap16(a):   # [NCORES*NT, n] -> [NCORES, 16, NT*n//16]
        n = a.shape[1]
        return (a.reshape(NCORES, NT, n // 16, 16).transpose(0, 3, 1, 2)
                .reshape(NCORES, 16, NT * n // 16))

    pidx = np.concatenate(
        [wrap16(idx[:, :nA]), wrap16(idx[:, nA:])],
        axis=2).reshape(NCORES * 16, -1)

    # pdl pack: dl columns then batch columns, [NCORES*128, NT*CH + NT] int8
    pdl = np.empty((NCORES, 128, NT * CH + NT), np.int8)
    pdl[:, :, :NT * CH] = (dlp.reshape(NCORES, NT, CH, 128)
                           .transpose(0, 3, 1, 2).reshape(NCORES, 128, NT * CH))
    bpad = np.zeros((NCORES, RPAD), np.int8)
    bpad[:, :RP] = batch.reshape(NCORES, RP)
    pdl[:, :, NT * CH:] = bpad.reshape(NCORES, NT, 128).transpose(0, 2, 1)
    cntg = np.bincount(batch.astype(np.int64), minlength=G).astype(np.float32)
    return pidx, pdl.reshape(NCORES * 128, -1), cntg


# ---------------------------------------------------------------- bass build

def _build_nc(CHA, CHB):
    from contextlib import ExitStack
    from concourse import bacc, mybir
    from concourse import tile

    F32 = mybir.dt.float32
    BF16 = mybir.dt.bfloat16
    FP8 = mybir.dt.float8e4
    I16 = mybir.dt.int16
    AF = mybir.ActivationFunctionType
    OP = mybir.AluOpType
    CH = CHA + CHB
    CI = NT * (CHA + CHB) * 8

    nc = bacc.Bacc(None, target_bir_lowering=False, debug=False)
    dp = nc.declare_dram_parameter
    I8 = mybir.dt.int8
    px8 = dp("px8", [128, RPAD], FP8, isOutput=False)
    pw = dp("pw", [16, WOFF["_total"]], BF16, isOutput=False)
    pdl = dp("pdl", [128, NT * CH + NT], I8, isOutput=False)
    pidx = dp("pidx", [16, CI], I16, isOutput=False)
    pw_loc = nc.dram_tensor("pw_loc", [16, WOFF["_total"]], BF16)
    pw_full = nc.dram_tensor("pw_full", [128, WOFF["_total"]], BF16,
                             addr_space="Shared")
    out_pool = dp("out_pool", [G, C], F32, isOutput=True)

    xl1_sl = nc.dram_tensor("xl1_sl", [RPAD, HC], BF16)
    xr1_loc = nc.dram_tensor("xr1_loc", [RPAD, HC], BF16)
    xl1_full = nc.dram_tensor("xl1_full", [NPADG, HC], BF16, addr_space="Shared")
    h1c_sl = [nc.dram_tensor(f"h1c{i}_sl", [RPAD, 128], BF16) for i in range(2)]
    xl2_sl = nc.dram_tensor("xl2_sl", [RPAD, C], F32)
    xr2_loc = nc.dram_tensor("xr2_loc", [RPAD, C], F32)
    xl2_full = nc.dram_tensor("xl2_full", [NPADG, C], F32, addr_space="Shared")
    pool_part = nc.dram_tensor("pool_part", [G, C], F32)
    pool_sum = nc.dram_tensor("pool_sum", [G, C], F32, addr_space="Shared")

    with tile.TileContext(nc) as tc, ExitStack() as ctx:
        cp = ctx.enter_context(tc.tile_pool(name="consts", bufs=1))
        sb = ctx.enter_context(tc.tile_pool(name="work", bufs=2))

        # weights ship as a per-core 16-row shard; device AllGather
        # reassembles the full [128, W] pack (0.36 MB over the host wire
        # instead of 2.9 MB of per-core replicas). Collectives cannot read
        # IO tensors, so bounce the shard through an internal DRAM tensor.
        pwt = cp.tile([16, WOFF["_total"]], BF16, tag="pwt")
        nc.sync.dma_start(pwt[:], pw[:, :])
        nc.sync.dma_start(pw_loc[:, :], pwt[:])
        nc.gpsimd.collective_compute(
            "AllGather", mybir.AluOpType.bypass,
            replica_groups=[list(range(NCORES))],
            ins=[pw_loc[:, :]], outs=[pw_full[:, :]])

        def cload(name, lo, hi, dtype=BF16):
            t = cp.tile([128, hi - lo], dtype, tag=name)
            nc.sync.dma_start(t[:], pw_full[:, lo:hi])
            return t

        x8 = cp.tile([128, RPAD], FP8, tag="x8c")
        nc.sync.dma_start(x8[:], px8[:, :])
        wl1_t = cload("wl1c", *WOFF["wl1"])
        wr1_t = cload("wr1c", *WOFF["wr1"])
        wl2_t = cload("wl2c", *WOFF["wl2"])          # [128, 2*C]
        wr2_t = cload("wr2c", *WOFF["wr2"])
        att_t = cload("attc", *WOFF["att1"])
        att2_t = cload("att2c", *WOFF["att2"])
        b1_b = cload("b1c", *WOFF["b1"])
        b2_b = cload("b2c", *WOFF["b2"])
        dl_t = cp.tile([128, NT * CH], I8, tag="dlc")
        nc.sync.dma_start(dl_t[:], pdl[:, :NT * CH])
        batch_t = cp.tile([128, NT], I8, tag="batchc")
        nc.sync.dma_start(batch_t[:], pdl[:, NT * CH:])

        b1_t = cp.tile([128, HC], F32)
        nc.vector.tensor_copy(b1_t[:], b1_b[:])
        b2_t = cp.tile([128, C], F32)
        nc.vector.tensor_copy(b2_t[:], b2_b[:])

        iota_i = cp.tile([128, 128], mybir.dt.int32)
        nc.gpsimd.iota(iota_i[:], pattern=[[1, 128]], base=0, channel_multiplier=0)
        iota_f = cp.tile([128, 128], I8)
        nc.vector.tensor_copy(iota_f[:], iota_i[:])

        nIA, nIB = NT * CHA * 8, NT * CHB * 8
        iA_t = cp.tile([128, nIA], I16)
        iB_t = cp.tile([128, nIB], I16)
        for k in range(8):
            nc.sync.dma_start(iA_t[16 * k:16 * (k + 1), :], pidx[:, 0:nIA])
            nc.sync.dma_start(iB_t[16 * k:16 * (k + 1), :], pidx[:, nIA:nIA + nIB])

        # xr gather rows, derived from dl on device: clamp(-1 -> 0), cast to
        # int16, then shuffle into the 16-row-wrap dma_gather index layout
        # (row p%16, col chunk*8 + p//16) and replicate into all 8 row groups.
        dl0 = cp.tile([128, NT * CH], I8)
        nc.vector.tensor_scalar(dl0[:], dl_t[:], 0, None, OP.max)
        dli = cp.tile([128, NT * CH], I16)
        nc.vector.tensor_copy(dli[:], dl0[:])
        iR3 = cp.tile([128, NT * CH, 8], I16)
        for q in range(8):
            nc.sync.dma_start(iR3[0:16, :, q], dli[16 * q:16 * (q + 1), :])
        for k in range(1, 8):
            nc.sync.dma_start(iR3[16 * k:16 * (k + 1), :, :], iR3[0:16, :, :])
        iR_t = iR3[:].rearrange("p a b -> p (a b)")

        # ---- stage 1: layer-1 node transforms (x fp8 -> bf16 per tile)
        psx = ExitStack()
        ctx.callback(psx.close)
        ps = psx.enter_context(tc.tile_pool(name="ps1", bufs=2, space="PSUM"))
        for i in range(NT):
            xs = sb.tile([128, 128], BF16, tag="xs")
            nc.vector.tensor_copy(xs[:], x8[:, i * 128:(i + 1) * 128])
            pa = ps.tile([128, HC], F32, tag="p_nl")
            nc.tensor.matmul(pa[:], xs[:], wl1_t[:], start=True, stop=True)
            ta = sb.tile([128, HC], BF16, tag="t_nl")
            nc.vector.tensor_add(ta[:], pa[:], b1_t[:])
            nc.sync.dma_start(xl1_sl[i * 128:(i + 1) * 128, :], ta[:])
            pb = ps.tile([128, HC], F32, tag="p_nr")
            nc.tensor.matmul(pb[:], xs[:], wr1_t[:], start=True, stop=True)
            tb = sb.tile([128, HC], BF16, tag="t_nr")
            nc.vector.tensor_sub(tb[:], pb[:], b1_t[:])
            nc.sync.dma_start(xr1_loc[i * 128:(i + 1) * 128, :], tb[:])

        nc.gpsimd.collective_compute(
            "AllGather", mybir.AluOpType.bypass,
            replica_groups=[list(range(NCORES))],
            ins=[xl1_sl[:, :]], outs=[xl1_full[:, :]])

        MAXCH = 8  # dma_gather is only safe up to 1024 indices per call

        def gathers(out3, in_ap, idx_t, col0, nch, elem):
            for b0 in range(0, nch, MAXCH):
                b1 = min(b0 + MAXCH, nch)
                n = (b1 - b0) * 128
                nc.gpsimd.dma_gather(
                    out3[:, b0:b1, :], in_ap,
                    idx_t[:, col0 + b0 * 8:col0 + b1 * 8],
                    num_idxs=n, num_idxs_reg=n, elem_size=elem)

        # ---- stage 2: layer-1 edge stage per dst tile
        psx.close()
        ps = psx.enter_context(tc.tile_pool(name="ps2", bufs=2, space="PSUM"))
        for t in range(NT):
            gxl = sb.tile([128, CH, HC], BF16, tag="gxl")
            gathers(gxl[:, 0:CHA, :], xl1_full[0:BANKA, :], iA_t,
                    t * CHA * 8, CHA, HC)
            gathers(gxl[:, CHA:CH, :], xl1_full[BANKA:NPADG, :], iB_t,
                    t * CHB * 8, CHB, HC)
            gxr = sb.tile([128, CH, HC], BF16, tag="gxr")
            gathers(gxr[:, :, :], xr1_loc[t * 128:(t + 1) * 128, :], iR_t,
                    t * CH * 8, CH, HC)

            ex_t = sb.tile([128, CH, H], BF16, tag="ex")
            ind_t = sb.tile([128, CH, 128], BF16, tag="ind")
            den_p = ps.tile([128, H], F32, tag="den")
            z = sb.tile([128, CH, HC], BF16, tag="z", bufs=1)
            nc.vector.tensor_add(z[:], gxl[:], gxr[:])
            r = sb.tile([128, CH, HC], BF16, tag="r", bufs=1)
            nc.scalar.activation(r[:], z[:], AF.Relu, scale=0.8)
            nc.scalar.mul(z[:], gxl[:], 0.2)
            nc.vector.tensor_add(r[:], r[:], z[:])
            nc.vector.tensor_tensor(
                z[:], r[:],
                att_t[:].unsqueeze(1).broadcast_to([128, CH, HC]), OP.mult)
            sc = sb.tile([128, CH, H], F32, tag="sc")
            nc.vector.tensor_reduce(
                sc[:], z[:].rearrange("p t (h c) -> p t h c", h=H),
                axis=mybir.AxisListType.X, op=OP.add)
            nc.scalar.activation(ex_t[:], sc[:], AF.Exp)
            nc.vector.tensor_tensor(
                ind_t[:], iota_f[:].unsqueeze(1).broadcast_to([128, CH, 128]),
                dl_t[:, t * CH:(t + 1) * CH].unsqueeze(2).broadcast_to(
                    [128, CH, 128]), OP.is_equal)
            for j in range(CH):
                nc.tensor.matmul(den_p[:], ind_t[:, j, :], ex_t[:, j, :],
                                 start=(j == 0), stop=(j == CH - 1))
            rden = sb.tile([128, H], F32, tag="rden")
            nc.vector.tensor_scalar(rden[:], den_p[:], 1e-20, None, OP.max)
            nc.vector.reciprocal(rden[:], rden[:])

            agg_p = ps.tile([128, HC], F32, tag="agg")
            msg = sb.tile([128, CH, HC], BF16, tag="msg", bufs=1)
            nc.vector.tensor_tensor(
                msg[:].rearrange("p t (h c) -> p t h c", h=H),
                gxl[:].rearrange("p t (h c) -> p t h c", h=H),
                ex_t[:].unsqueeze(3).broadcast_to([128, CH, H, C]), OP.mult)
            for j in range(CH):
                nc.tensor.matmul(agg_p[:], ind_t[:, j, :], msg[:, j, :],
                                 start=(j == 0), stop=(j == CH - 1))
            h1_t = sb.tile([128, HC], BF16, tag="h1")
            for h in range(H):
                nc.scalar.activation(h1_t[:, h * C:(h + 1) * C],
                                     agg_p[:, h * C:(h + 1) * C],
                                     AF.Relu, scale=rden[:, h:h + 1])
            for i in range(2):
                nc.sync.dma_start(h1c_sl[i][t * 128:(t + 1) * 128, :],
                                  h1_t[:, i * 128:(i + 1) * 128])

        # ---- stage 3: layer-2 node transforms
        psx.close()
        ps = psx.enter_context(tc.tile_pool(name="ps3", bufs=2, space="PSUM"))
        h1T = cp.tile([128, 2, RPAD], BF16)
        for i in range(2):
            nc.sync.dma_start_transpose(h1T[:, i, :], h1c_sl[i][:, :])
        for i in range(NT):
            pa = ps.tile([128, C], F32, tag="p2_nl")
            for cc in range(2):
                nc.tensor.matmul(pa[:], h1T[:, cc, i * 128:(i + 1) * 128],
                                 wl2_t[:, cc * C:(cc + 1) * C],
                                 start=(cc == 0), stop=(cc == 1))
            ta = sb.tile([128, C], F32, tag="t2_nl")
            nc.vector.tensor_add(ta[:], pa[:], b2_t[:])
            nc.sync.dma_start(xl2_sl[i * 128:(i + 1) * 128, :], ta[:])
            pb = ps.tile([128, C], F32, tag="p2_nr")
            for cc in range(2):
                nc.tensor.matmul(pb[:], h1T[:, cc, i * 128:(i + 1) * 128],
                                 wr2_t[:, cc * C:(cc + 1) * C],
                                 start=(cc == 0), stop=(cc == 1))
            tb = sb.tile([128, C], F32, tag="t2_nr")
            nc.vector.tensor_sub(tb[:], pb[:], b2_t[:])
            nc.sync.dma_start(xr2_loc[i * 128:(i + 1) * 128, :], tb[:])

        nc.gpsimd.collective_compute(
            "AllGather", mybir.AluOpType.bypass,
            replica_groups=[list(range(NCORES))],
            ins=[xl2_sl[:, :]], outs=[xl2_full[:, :]])

        # ---- stage 4: layer-2 edge stage + pooling
        psx.close()
        ps = psx.enter_context(tc.tile_pool(name="ps4", bufs=2, space="PSUM"))
        pool_acc = cp.tile([G, C], F32)
        nc.vector.memset(pool_acc[:], 0.0)
        for t in range(NT):
            gxl2 = sb.tile([128, CH, C], F32, tag="gxl2")
            gathers(gxl2[:, 0:CHA, :], xl2_full[0:BANKA, :], iA_t,
                    t * CHA * 8, CHA, C)
            gathers(gxl2[:, CHA:CH, :], xl2_full[BANKA:NPADG, :], iB_t,
                    t * CHB * 8, CHB, C)
            gxr2 = sb.tile([128, CH, C], F32, tag="gxr2")
            gathers(gxr2[:, :, :], xr2_loc[t * 128:(t + 1) * 128, :], iR_t,
                    t * CH * 8, CH, C)

            ex2_t = sb.tile([128, CH, 1], BF16, tag="ex2")
            ind2_t = sb.tile([128, CH, 128], BF16, tag="ind2")
            den2_p = ps.tile([128, 1], F32, tag="den2")
            z = sb.tile([128, CH, C], BF16, tag="z2", bufs=1)
            nc.vector.tensor_add(z[:], gxl2[:], gxr2[:])
            r = sb.tile([128, CH, C], BF16, tag="r2", bufs=1)
            nc.scalar.activation(r[:], z[:], AF.Relu, scale=0.8)
            nc.scalar.mul(z[:], gxl2[:], 0.2)
            nc.vector.tensor_add(r[:], r[:], z[:])
            nc.vector.tensor_tensor(
                z[:], r[:],
                att2_t[:].unsqueeze(1).broadcast_to([128, CH, C]), OP.mult)
            sc = sb.tile([128, CH, 1], F32, tag="sc2")
            nc.vector.tensor_reduce(
                sc[:], z[:].unsqueeze(2),
                axis=mybir.AxisListType.X, op=OP.add)
            nc.scalar.activation(ex2_t[:], sc[:], AF.Exp)
            nc.vector.tensor_tensor(
                ind2_t[:], iota_f[:].unsqueeze(1).broadcast_to([128, CH, 128]),
                dl_t[:, t * CH:(t + 1) * CH].unsqueeze(2).broadcast_to(
                    [128, CH, 128]), OP.is_equal)
            for j in range(CH):
                nc.tensor.matmul(den2_p[:], ind2_t[:, j, :], ex2_t[:, j, :],
                                 start=(j == 0), stop=(j == CH - 1))
            rden2 = sb.tile([128, 1], F32, tag="rden2")
            nc.vector.tensor_scalar(rden2[:], den2_p[:], 1e-20, None, OP.max)
            nc.vector.reciprocal(rden2[:], rden2[:])

            agg2_p = ps.tile([128, C], F32, tag="agg2")
            msg = sb.tile([128, CH, C], BF16, tag="msg2", bufs=1)
            nc.vector.tensor_tensor(
                msg[:], gxl2[:],
                ex2_t[:].broadcast_to([128, CH, C]), OP.mult)
            for j in range(CH):
                nc.tensor.matmul(agg2_p[:], ind2_t[:, j, :], msg[:, j, :],
                                 start=(j == 0), stop=(j == CH - 1))
            h2_t = sb.tile([128, C], BF16, tag="h2")
            nc.scalar.mul(h2_t[:], agg2_p[:], rden2[:, 0:1])

            indp = sb.tile([128, G], BF16, tag="indp")
            nc.vector.tensor_tensor(
                indp[:], iota_f[:, 0:G],
                batch_t[:, t:t + 1].broadcast_to([128, G]), OP.is_equal)
            pool_p = ps.tile([G, C], F32, tag="poolp")
            nc.tensor.matmul(pool_p[:], indp[:], h2_t[:],
                             start=True, stop=True)
            nc.vector.tensor_add(pool_acc[:], pool_acc[:], pool_p[:])

        ot = cp.tile([G, C], F32)
        nc.vector.tensor_copy(ot[:], pool_acc[:])
        nc.sync.dma_start(pool_part[:, :], ot[:])
        nc.gpsimd.collective_compute(
            "AllReduce", mybir.AluOpType.add,
            replica_groups=[list(range(NCORES))],
            ins=[pool_part[:, :]], outs=[pool_sum[:, :]])
        nc.sync.dma_start(out_pool[:, :], pool_sum[:, :])

    nc.finalize()
    return nc


# ------------------------------------------------------------ cached runner

class _Runner:
    """Holds the Bass module, a persistently-jitted shard_map callable,
    and the device mesh, so repeat kernel() calls skip all tracing,
    lowering, and compilation."""

    def __init__(self, CHA, CHB):
        import jax
        from jax.sharding import Mesh, PartitionSpec, NamedSharding
        from jax.experimental.shard_map import shard_map
        from concourse import bass2jax, mybir

        self.jax = jax
        self.CHA, self.CHB = CHA, CHB
        nc = _build_nc(CHA, CHB)
        bass2jax.install_neuronx_cc_hook()

        partition_name = (nc.partition_id_tensor.name
                          if nc.partition_id_tensor else None)
        in_names, out_names, out_avals, zero_shapes = [], [], [], []
        for alloc in nc.m.functions[0].allocations:
            if not isinstance(alloc, mybir.MemoryLocationSet):
                continue
            name = alloc.memorylocations[0].name
            if alloc.kind == "ExternalInput":
                if name != partition_name:
                    in_names.append(name)
            elif alloc.kind == "ExternalOutput":
                shape = tuple(alloc.tensor_shape)
                dtype = mybir.dt.np(alloc.dtype)
                out_names.append(name)
                out_avals.append(jax.core.ShapedArray(shape, dtype))
                zero_shapes.append((shape, dtype))
        n_params = len(in_names)
        all_names = list(in_names) + list(out_names)
        if partition_name is not None:
            all_names.append(partition_name)

        def _body(*args):
            operands = list(args)
            if partition_name is not None:
                operands.append(bass2jax.partition_id_tensor())
            outs = bass2jax._bass_exec_p.bind(
                *operands,
                out_avals=tuple(out_avals),
                in_names=tuple(all_names),
                out_names=tuple(out_names),
                lowering_input_output_aliases=(),
                sim_require_finite=True,
                sim_require_nnan=True,
                nc=nc,
            )
            return tuple(outs)

        self.devices = jax.devices()[:NCORES]
        assert len(self.devices) == NCORES
        mesh = Mesh(np.asarray(self.devices), ("core",))
        self.sharding = NamedSharding(mesh, PartitionSpec("core"))
        n_outs = len(out_names)
        # No donation: the kernel writes every output element, so the
        # uninitialized custom-call result buffers are fine, and the zero
        # "output-seed" inputs can live on device permanently.
        self.fn = jax.jit(
            shard_map(_body, mesh=mesh,
                      in_specs=(PartitionSpec("core"),) * (n_params + n_outs),
                      out_specs=(PartitionSpec("core"),) * n_outs,
                      check_rep=False),
            keep_unused=True)
        self.in_names = in_names
        self.out_names = out_names
        self.zero_shapes = zero_shapes
        self.dev_zeros = [
            self.start_put(np.zeros((NCORES * s[0], *s[1:]), d))()
            for s, d in zero_shapes]
        # XLA-vectorized f32->fp8 cast on the CPU backend: ~6ms for x vs
        # ~52ms through ml_dtypes' scalar loop
        import jax.numpy as jnp
        self.cast8 = jax.jit(lambda a: a.astype(jnp.float8_e4m3),
                             backend="cpu")
        np.asarray(self.cast8(np.zeros((N, F_IN), np.float32)))

    def start_put(self, arr=None, shape=None, dtype=None, produce=None):
        """Async per-device sharded transfer: device_put dispatch only (the
        relay's own IO threads move the bytes), global array assembled from
        the unready buffers. No client-side ack round-trip — execution is
        sequenced after the transfers server-side; the only blocking await
        in a call is the final result fetch. Returns a handle for symmetry
        with the old threaded API."""
        jax = self.jax
        if arr is not None:
            shape = arr.shape
            d0 = shape[0] // NCORES
            per = arr.reshape(NCORES, d0, *shape[1:])
            produce = lambda i: per[i]
        bufs = [jax.device_put(np.ascontiguousarray(produce(i)),
                               self.devices[i])
                for i in range(NCORES)]
        garr = jax.make_array_from_single_device_arrays(
            shape, self.sharding, bufs)
        return lambda: garr

    def run_handles(self, handles):
        """handles: name -> handle from start_put. Returns the [G, C]
        pooled sum (identical on every core after the on-device AllReduce;
        only core 0's shard is pulled back)."""
        args = [handles[n]() for n in self.in_names]
        outs = self.fn(*args, *self.dev_zeros)
        return np.asarray(outs[0].addressable_shards[0].data)


_RUNNERS = {}


def _get_runner(CHA, CHB):
    key = (CHA, CHB)
    if key not in _RUNNERS:
        _RUNNERS[key] = _Runner(CHA, CHB)
    return _RUNNERS[key]


def _warmup():
    r = _get_runner(CHA0, CHB0)
    CH = CHA0 + CHB0
    CI = NT * (CHA0 + CHB0) * 8
    handles = {
        "px8": r.start_put(np.zeros((NCORES * 128, RPAD), F8)),
        "pw": r.start_put(np.zeros((NCORES * 16, WOFF["_total"]), BF)),
        "pdl": r.start_put(np.zeros((NCORES * 128, NT * CH + NT), np.int8)),
        "pidx": r.start_put(np.zeros((NCORES * 16, CI), np.int16)),
    }
    r.run_handles(handles)
    return r


try:
    _warmup()
except Exception:
    _RUNNERS.clear()


# -------------------------------------------------------------------- driver

def kernel(x, edge_index, batch, Wl1, Wr1, att1, b1, Wl2, Wr2, att2, b2,
           Wo, bo):
    x = np.asarray(x, np.float32)
    edge_index = np.asarray(edge_index)
    batch = np.asarray(batch)
    Wl1 = np.asarray(Wl1, np.float32); Wr1 = np.asarray(Wr1, np.float32)
    att1 = np.asarray(att1, np.float32); b1 = np.asarray(b1, np.float32)
    Wl2 = np.asarray(Wl2, np.float32); Wr2 = np.asarray(Wr2, np.float32)
    att2 = np.asarray(att2, np.float32); b2 = np.asarray(b2, np.float32)
    Wo = np.asarray(Wo, np.float32); bo = np.asarray(bo, np.float32)

    CHA, CHB = CHA0, CHB0
    runner = _RUNNERS.get((CHA, CHB)) or _get_runner(CHA, CHB)

    # weight pack is tiny and preprocessing-independent: fill + ship first
    # so the wire is busy during the fp8 cast of x
    pwh = np.empty((128, WOFF["_total"]), BF)

    def put(name, a):
        lo, hi = WOFF[name]
        pwh[:, lo:hi] = a.astype(BF)

    put("wl1", Wl1); put("wr1", Wr1)
    put("wl2", Wl2.reshape(2, 128, C).transpose(1, 0, 2).reshape(128, 2 * C))
    put("wr2", Wr2.reshape(2, 128, C).transpose(1, 0, 2).reshape(128, 2 * C))
    put("att1", np.broadcast_to(att1.reshape(1, HC), (128, HC)))
    put("att2", np.broadcast_to(att2.reshape(1, C), (128, C)))
    put("b1", np.broadcast_to(b1.reshape(1, HC), (128, HC)))
    put("b2", np.broadcast_to(b2.reshape(1, C), (128, C)))
    h_w = runner.start_put(pwh)   # [128, W]: 16-row shard per core

    # x: one XLA-vectorized fp8 cast (~6ms), then per-core byte transposes
    # into the padded shards; each core's transfer dispatches immediately
    y8 = np.asarray(runner.cast8(x)).reshape(NCORES, RP, F_IN)

    def make_px8(i):
        b = np.empty((128, RPAD), F8)
        b[:, :RP] = y8[i].T
        b[:, RP:] = 0.0
        return b

    h_x = runner.start_put(shape=(NCORES * 128, RPAD), dtype=F8,
                           produce=make_px8)

    pre = _preprocess(edge_index, batch, CHA, CHB)
    while pre is None:  # capacity overflow: grow and rebuild (cold path)
        CHA += 2; CHB += 2
        pre = _preprocess(edge_index, batch, CHA, CHB)
        runner = _get_runner(CHA, CHB)
    pidx, pdl, cntg = pre
    h_idx = runner.start_put(pidx)
    h_dl = runner.start_put(pdl)

    pooled = runner.run_handles(dict(px8=h_x, pw=h_w, pdl=h_dl, pidx=h_idx))
    pooled = pooled / np.maximum(cntg, 1.0)[:, None]
    return (pooled @ Wo + bo).astype(np.float32)
